# revision 3
# baseline (speedup 1.0000x reference)
"""Trainium2 Bass kernel v2 for nn_DilatedMHCABlock.

Same mod-4 subsequence decomposition as the baseline (16 subsequences of 512
tokens with +-8 banded attention, 2 per core, zero collectives), rebuilt
around HW-profile findings:

- bf16 everywhere (f32 PSUM accumulation): halves DVE element cost, halves
  DMA bytes, keeps matmul at 1 cycle/row at any free size.
- Both Q AND K are norm-folded before scores, so exp needs no per-partition
  scale and one [128, 768] activation covers B|C|edge chunks per head.
- One [128, 768] mask multiply per head (masks are host constants).
- AV writes head pairs directly at PSUM partition offsets 0/64 (legal
  offsets are 0/32/64) - no partition-staging DMAs or copies.
- Softmax denominators accumulate into a shared [16, 512] PSUM tile via
  per-head indicator matmuls; normalization is deferred past AV.
- ~40 large contiguous DMAs (vs 245 small ones): weights are host-packed so
  every DMA descriptor is >= 1KB contiguous.
- Scores for iteration k+1 are emitted before AV of iteration k so the
  in-order PE queue never stalls on the exp/mask round trip.
"""
import sys

sys.path.insert(0, "/opt/trn_rl_repo")

import numpy as np

import bass_rust
import concourse.bass as bass
import concourse.mybir as mybir
import concourse.tile as tile

F32 = mybir.dt.float32
BF16 = mybir.dt.bfloat16
EPS = 1e-6
N_CORES = 8
ACT = mybir.ActivationFunctionType


# ---------------------------------------------------------------------------
# walrus wait legalization (same as baseline): at most 1 sync wait per
# instruction (2 on EventSemaphore); split the excess.
_wait_counter = [0]


def _legalize_waits(nc):
    f = nc.m.functions[0]
    for blk in f.blocks:
        insts = blk.instructions
        out = []
        changed = False
        for inst in insts:
            si = inst.sync_info
            waits = list(si.on_wait) if si is not None else []
            cap = 2 if isinstance(inst, mybir.InstEventSemaphore) else 1
            if len(waits) > cap:
                extra, keep = waits[:-cap], waits[-cap:]
                for i in range(0, len(extra), 2):
                    es = mybir.InstEventSemaphore(
                        name=f"wait_split_{_wait_counter[0]}", ins=[], outs=[]
                    )
                    _wait_counter[0] += 1
                    es.engine = inst.engine
                    es.sync_info = bass_rust.SyncInfo(
                        on_wait=extra[i : i + 2], on_update=[]
                    )
                    out.append(es)
                si.on_wait = keep
                changed = True
            out.append(inst)
        if changed:
            blk.instructions = out


# ---------------------------------------------------------------------------
def _build_nc(phases=5):
    nc = bass.Bass()

    xT_d = nc.declare_dram_parameter("xT", [128, 8, 1024], BF16, isOutput=False)
    wq_d = nc.declare_dram_parameter("wq", [8, 128, 8, 128], BF16, isOutput=False)
    wk_d = nc.declare_dram_parameter("wk", [8, 128, 8, 128], BF16, isOutput=False)
    wo_d = nc.declare_dram_parameter("wo", [8, 128, 8, 128], BF16, isOutput=False)
    wv_d = nc.declare_dram_parameter("wv", [2, 128, 8, 512], BF16, isOutput=False)
    bq_d = nc.declare_dram_parameter("bq", [128, 8], F32, isOutput=False)
    bk_d = nc.declare_dram_parameter("bk", [128, 8], F32, isOutput=False)
    bo_d = nc.declare_dram_parameter("bo", [128, 8], F32, isOutput=False)
    mask_d = nc.declare_dram_parameter("masks", [128, 1024], BF16, isOutput=False)
    i16_d = nc.declare_dram_parameter("ind16", [128, 8, 16], BF16, isOutput=False)
    i16t_d = nc.declare_dram_parameter("ind16T", [16, 8, 128], BF16, isOutput=False)
    sind_d = nc.declare_dram_parameter("sind", [128, 16, 16], BF16, isOutput=False)
    out_d = nc.declare_dram_parameter("outT", [128, 8, 1024], BF16, isOutput=True)

    with tile.TileContext(nc) as tc, nc.allow_low_precision(
        reason="bf16 compute within 2e-2 tolerance"
    ):
        _emit(nc, tc, xT_d, wq_d, wk_d, wo_d, wv_d, bq_d, bk_d, bo_d, mask_d,
              i16_d, i16t_d, sind_d, out_d, phases)

    _legalize_waits(nc)
    return nc


def _emit(nc, tc, xT_d, wq_d, wk_d, wo_d, wv_d, bq_d, bk_d, bo_d, mask_d,
          i16_d, i16t_d, sind_d, out_d, phases=5):
    from contextlib import ExitStack

    ctx = ExitStack()
    with ctx:
        p_const = ctx.enter_context(tc.tile_pool(name="const", bufs=1))
        p_x = ctx.enter_context(tc.tile_pool(name="x", bufs=1))
        p_w = ctx.enter_context(tc.tile_pool(name="w", bufs=2))
        p_wo = ctx.enter_context(tc.tile_pool(name="wo", bufs=8))
        p_wv = ctx.enter_context(tc.tile_pool(name="wv", bufs=2))
        p_qk = ctx.enter_context(tc.tile_pool(name="qk", bufs=32))
        p_v = ctx.enter_context(tc.tile_pool(name="v", bufs=8))
        p_at = ctx.enter_context(tc.tile_pool(name="at", bufs=16))
        p_sq = ctx.enter_context(tc.tile_pool(name="sq", bufs=16))
        p_inv = ctx.enter_context(tc.tile_pool(name="inv", bufs=10))
        p_ex = ctx.enter_context(tc.tile_pool(name="ex", bufs=3))
        p_ot = ctx.enter_context(tc.tile_pool(name="ot", bufs=2))
        pp = ctx.enter_context(tc.tile_pool(name="pp", bufs=1, space="PSUM"))
        scb = [pp.tile([128, 1024], F32, tag=f"scb{r}", name=f"scb{r}")
               for r in range(3)]
        sml = pp.tile([128, 1024], F32, tag="sml", name="sml")
        psn = [0]  # rolling counter for [128, 512] matmul psum regions

        def next_ps():
            r = scb[psn[0] % 3][:, 0:512]
            psn[0] += 1
            return r

        # ---- constants ----------------------------------------------------
        bq_sb = p_const.tile([128, 8], F32, tag="bq")
        bk_sb = p_const.tile([128, 8], F32, tag="bk")
        bo_sb = p_const.tile([128, 8], F32, tag="bo")
        nc.gpsimd.dma_start(out=bq_sb, in_=bq_d[:, :])
        nc.gpsimd.dma_start(out=bk_sb, in_=bk_d[:, :])
        nc.gpsimd.dma_start(out=bo_sb, in_=bo_d[:, :])

        m_sb = p_const.tile([128, 1024], BF16, tag="masks")
        nc.gpsimd.dma_start(out=m_sb, in_=mask_d[:, :])
        zq = p_const.tile([128, 64], BF16, tag="zq")
        nc.vector.memset(zq, 0.0)
        ind16 = p_const.tile([128, 8, 16], BF16, tag="ind16")
        nc.gpsimd.dma_start(out=ind16, in_=i16_d[:, :, :])
        ind16T = p_const.tile([16, 8, 128], BF16, tag="ind16T")
        nc.gpsimd.dma_start(out=ind16T, in_=i16t_d[:, :, :])
        sind = p_const.tile([128, 16, 16], BF16, tag="sind")
        nc.gpsimd.dma_start(out=sind, in_=sind_d[:, :, :])

        xT = p_x.tile([128, 8, 1024], BF16, tag="xT")
        for j in range(8):
            nc.scalar.dma_start(out=xT[:, j, :], in_=xT_d[:, j, :])

        # ---- Q/K projections + bias --------------------------------------
        qt = [[None, None] for _ in range(8)]
        kt = [[None, None] for _ in range(8)]
        for w_d, b_sb, dst in ((wk_d, bk_sb, kt), (wq_d, bq_sb, qt)):
            for i in range(8):
                wt = p_w.tile([128, 8, 128], BF16, tag="w")
                nc.sync.dma_start(out=wt, in_=w_d[i])
                ps = [next_ps() for c in range(2)]
                for j in range(8):
                    for c in range(2):
                        nc.tensor.matmul(
                            ps[c], wt[:, j, :],
                            xT[:, j, 512 * c : 512 * c + 512],
                            start=(j == 0), stop=(j == 7),
                        )
                for c in range(2):
                    t = p_qk.tile([128, 512], BF16, tag="qk", name="qk")
                    nc.vector.tensor_scalar_add(t, ps[c], b_sb[:, i : i + 1])
                    dst[i][c] = t

        # ---- norms: squares + reductions ---------------------------------
        # squares: Q on DVE, K on Act (engine balance)
        sqq = [[None, None] for _ in range(8)]
        sqk = [[None, None] for _ in range(8)]
        for i in range(8):
            for c in range(2):
                tq = p_sq.tile([128, 512], BF16, tag="sq", name="tq")
                nc.vector.tensor_mul(tq, qt[i][c], qt[i][c])
                sqq[i][c] = tq
                tk = p_sq.tile([128, 512], BF16, tag="sqk", name="tk")
                nc.scalar.square(tk, kt[i][c])
                sqk[i][c] = tk
        nrm_regions = {
            ("k", 0): sml[64:80, 512:1024],
            ("k", 1): sml[64:80, 0:512],
            ("q", 0): sml[0:16, 512:1024],
            ("q", 1): sml[32:48, 512:1024],
        }
        nrm_ps = {}
        for nm, sq in (("k", sqk), ("q", sqq)):
            for c in range(2):
                p2 = nrm_regions[(nm, c)]
                for i in range(8):
                    nc.tensor.matmul(p2, ind16[:, i, :], sq[i][c],
                                     start=(i == 0), stop=(i == 7))
                nrm_ps[(nm, c)] = p2

        # 1/||v|| = exp(-0.5 * ln(||v||^2)); values are O(10), eps-free
        inv = {}
        for nm in ("k", "q"):
            for c in range(2):
                t = p_inv.tile([16, 512], F32, tag="nrmf", name=f"nf{nm}{c}")
                nc.scalar.activation(out=t, in_=nrm_ps[(nm, c)], func=ACT.Ln)
                tb = p_inv.tile([16, 512], BF16, tag="nrmb", name=f"nb{nm}{c}")
                nc.scalar.activation(out=tb, in_=t, func=ACT.Exp, scale=-0.5)
                inv[(nm, c)] = tb

        # ---- V projection interleaved with norm folding -------------------
        # V-proj matmul groups fill the PE while the DVE chews through the
        # bcast/fold chain, so the fold latency is hidden.
        v = [None] * 8
        wvt = [None, None]
        for co in range(2):
            wvt[co] = p_wv.tile([128, 8, 512], BF16, tag="wv", name=f"wvt{co}")
            nc.sync.dma_start(out=wvt[co], in_=wv_d[co])
        folds = [(c, i, nm) for c in range(2) for i in range(8)
                 for nm in ("k", "q")]
        fold_regions = [sml[:, 0:512], sml[:, 512:1024]]
        fold_n = [0]

        def emit_fold():
            if fold_n[0] >= len(folds):
                return
            c, i, nm = folds[fold_n[0]]
            pb = fold_regions[fold_n[0] % 2]
            fold_n[0] += 1
            dst = kt if nm == "k" else qt
            nc.tensor.matmul(pb, ind16T[:, i, :], inv[(nm, c)],
                             start=True, stop=True)
            nc.vector.tensor_mul(dst[i][c], dst[i][c], pb)

        psv_pend = []
        for g in range(8):
            for co in range(2):
                psv = next_ps()
                for j in range(8):
                    nc.tensor.matmul(
                        psv, xT[:, j, 128 * g : 128 * g + 128],
                        wvt[co][:, j, :], start=(j == 0), stop=(j == 7),
                    )
                emit_fold()
                emit_fold()
                psv_pend.append((g, co, psv))
                if v[g] is None:
                    v[g] = p_v.tile([128, 1024], BF16, tag="v", name=f"v{g}")
                # drain psv copies one group behind to keep psum slots free
                while len(psv_pend) > 2:
                    gg, cc2, pv = psv_pend.pop(0)
                    nc.vector.tensor_copy(
                        out=v[gg][:, 512 * cc2 : 512 * cc2 + 512], in_=pv
                    )
        while psv_pend:
            gg, cc2, pv = psv_pend.pop(0)
            nc.vector.tensor_copy(
                out=v[gg][:, 512 * cc2 : 512 * cc2 + 512], in_=pv
            )
        while fold_n[0] < len(folds):
            emit_fold()

        if phases <= 2:
            ot = p_ot.tile([128, 1024], BF16, tag="ot")
            nc.vector.tensor_copy(out=ot[:, 0:512], in_=qt[0][0])
            nc.vector.tensor_copy(out=ot[:, 512:1024], in_=kt[0][0])
            nc.gpsimd.dma_start(out=out_d[:, 0, :], in_=ot)
            return

        # ---- attention -----------------------------------------------------
        at = [[None, None] for _ in range(8)]
        for ht in range(8):
            for s in range(2):
                at[ht][s] = p_at.tile([128, 512], BF16, tag="at",
                                      name=f"at{ht}_{s}")
        sums_ps = [sml[0:16, 0:512], sml[32:48, 0:512]]

        iters = [(s, h) for s in range(2) for h in range(16)]
        from collections import deque

        # deferred normalization for subsequence s: 1/sum = exp(-ln(sum)),
        # then per-head-pair broadcast + multiply, threaded into the s=1
        # attention stream.
        inv_s = [None, None]

        def emit_inv_s(s):
            tf = p_inv.tile([16, 512], F32, tag="nrmf", name=f"sdf{s}")
            nc.scalar.activation(out=tf, in_=sums_ps[s], func=ACT.Ln)
            tb = p_inv.tile([16, 512], BF16, tag="nrmb", name=f"sdb{s}")
            nc.scalar.activation(out=tb, in_=tf, func=ACT.Exp, scale=-1.0)
            inv_s[s] = tb

        pbn_n = [0]

        def emit_pbn(ht, s):
            # alternate regions so the bcast->mul chain pipelines 2-deep;
            # scb[2] halves are the po slots, free once s=0 copies are done
            # (s=0 pbn) / all copies done (s=1 pbn)
            pbn = (sml[:, 512:1024] if (s == 0 or pbn_n[0] % 2 == 0)
                   else scb[2][:, 0:512])
            pbn_n[0] += 1
            nc.tensor.matmul(pbn, ind16T[:, ht, :], inv_s[s],
                             start=True, stop=True)
            nc.vector.tensor_mul(at[ht][s], at[ht][s], pbn)

        wo_t = [None] * 8

        def emit_oproj(i, c):
            if wo_t[i] is None:
                wo_t[i] = p_wo.tile([128, 8, 128], BF16, tag="wo",
                                    name=f"wo{i}")
                nc.sync.dma_start(out=wo_t[i], in_=wo_d[i])
            ps = next_ps()
            for j in range(8):
                nc.tensor.matmul(ps, wo_t[i][:, j, :], at[j][c],
                                 start=(j == 0), stop=(j == 7))
            ot = p_ot.tile([128, 512], BF16, tag="ot", name="ot")
            nc.scalar.activation(out=ot, in_=ps, func=ACT.Identity,
                                 bias=bo_sb[:, i : i + 1])
            nc.gpsimd.dma_start(
                out=out_d[:, i, 512 * c : 512 * c + 512], in_=ot
            )

        po_slot = [scb[2][:, 0:512], scb[2][:, 512:1024]]

        WINS = (0, 72, 200, 256)

        def emit_av(k2):
            s, h = iters[k2]
            hp = 64 * (h % 2)
            ht = h // 2
            ex = ex_pend[k2]
            po = po_slot[(k2 // 2) % 2]
            # zero-weight matmul initializes this head's 64-row stripe
            nc.tensor.matmul(po[hp : hp + 64, :], zq, ex[:, 0:512],
                             start=True, stop=True)
            for c in range(4):
                w = WINS[c]
                nc.tensor.matmul(
                    po[hp : hp + 64, w : w + 256],
                    v[4 * s + c][:, 64 * h : 64 * h + 64],
                    ex[:, 256 * c : 256 * c + 256],
                    start=False, stop=True,
                )
            if h == 0:
                nc.tensor.matmul(sums_ps[s], zq[:, 0:16], ex[:, 0:512],
                                 start=True, stop=True)
            for c in range(4):
                w = WINS[c]
                nc.tensor.matmul(
                    sums_ps[s][:, w : w + 256], sind[:, h, :],
                    ex[:, 256 * c : 256 * c + 256],
                    start=False, stop=True,
                )
            if hp == 64:
                nc.vector.tensor_copy(out=at[ht][s], in_=po)

        ex_pend = {}
        for k, (s, h) in enumerate(iters):
            hp = 64 * (h % 2)
            ht = h // 2
            scf = scb[k % 2]
            for c in range(4):
                w = WINS[c]
                nc.tensor.matmul(
                    scf[:, 256 * c : 256 * c + 256],
                    kt[ht][s][hp : hp + 64, 128 * c : 128 * c + 128],
                    qt[ht][s][hp : hp + 64, w : w + 256],
                    start=True, stop=True, tile_position=(hp, 0),
                )
            ex = p_ex.tile([128, 1024], BF16, tag="ex", name="ex")
            nc.scalar.activation(out=ex, in_=scf, func=ACT.Exp)
            nc.vector.tensor_mul(ex, ex, m_sb)
            ex_pend[k] = ex
            if k >= 2:
                emit_av(k - 2)
                del ex_pend[k - 2]
            if phases > 3:
                if k == 18:
                    emit_inv_s(0)
                elif 19 <= k <= 22:
                    emit_pbn(2 * (k - 19), 0)
                    emit_pbn(2 * (k - 19) + 1, 0)
        emit_av(30)
        emit_av(31)

        if phases <= 3:
            ot = p_ot.tile([128, 1024], BF16, tag="ot3", name="ot3")
            nc.vector.tensor_copy(out=ot[:, 0:512], in_=at[0][0])
            nc.vector.tensor_copy(out=ot[:, 512:1024], in_=at[0][1])
            nc.gpsimd.dma_start(out=out_d[:, 0, :], in_=ot)
            return

        # ---- epilogue: s=1 normalization, then both O projection halves ---
        emit_inv_s(1)
        for ht in range(8):
            emit_pbn(ht, 1)
        for i in range(8):
            emit_oproj(i, 0)
            emit_oproj(i, 1)


# ---------------------------------------------------------------------------
def _build_masks():
    # [128, 1024]: 4 blocks of 256 query-columns (windows 0/72/200/256).
    k = np.arange(128)[:, None]
    col = np.arange(256)[None, :]
    P0 = (np.abs(k - col) <= 8)
    P1 = (np.abs(k + 56 - col) <= 8)
    P3 = (np.abs(k + 128 - col) <= 8)
    return np.concatenate([P0, P1, P1, P3], axis=1).astype(np.float32)


_NC_CACHE = {}


def _get_nc(phases=5):
    if phases not in _NC_CACHE:
        _NC_CACHE[phases] = _build_nc(phases)
    return _NC_CACHE[phases]


def _bf16(x):
    import ml_dtypes

    return np.asarray(x, dtype=ml_dtypes.bfloat16)


def _make_in_maps(inputs, n_cores):
    x = np.asarray(inputs["x"], dtype=np.float32)
    Wq = np.asarray(inputs["Wq"], dtype=np.float32)
    Wk = np.asarray(inputs["Wk"], dtype=np.float32)
    Wv = np.asarray(inputs["Wv"], dtype=np.float32)
    Wo = np.asarray(inputs["Wo"], dtype=np.float32)
    bq = np.asarray(inputs["bq"], dtype=np.float32)
    bk = np.asarray(inputs["bk"], dtype=np.float32)
    bv = np.asarray(inputs["bv"], dtype=np.float32)
    bo = np.asarray(inputs["bo"], dtype=np.float32)

    def chunk_w(WT):  # [in, out] -> [8 i, 128 p, 8 j, 128 o]
        return np.ascontiguousarray(
            WT.reshape(8, 128, 8, 128).transpose(2, 1, 0, 3)
        )

    wq = _bf16(chunk_w(Wq.T))
    wk = _bf16(chunk_w(Wk.T))
    wo = _bf16(chunk_w(Wo.T))
    wv = _bf16(
        np.ascontiguousarray(Wv.T.reshape(8, 128, 2, 512).transpose(2, 1, 0, 3))
    )
    bo_eff = (bo + Wo @ bv).astype(np.float32)
    bq_a = np.ascontiguousarray(bq.reshape(8, 128).T)
    bk_a = np.ascontiguousarray(bk.reshape(8, 128).T)
    bo_a = np.ascontiguousarray(bo_eff.reshape(8, 128).T)

    masks = _bf16(_build_masks())
    ind16 = np.zeros((128, 8, 16), np.float32)
    for i in range(8):
        ind16[0:64, i, 2 * i] = 1.0
        ind16[64:128, i, 2 * i + 1] = 1.0
    ind16T = np.zeros((16, 8, 128), np.float32)
    for i in range(8):
        ind16T[2 * i, i, 0:64] = 1.0
        ind16T[2 * i + 1, i, 64:128] = 1.0
    sind = np.zeros((128, 16, 16), np.float32)
    for h in range(16):
        sind[:, h, h] = 1.0
    ind16 = _bf16(ind16)
    ind16T = _bf16(ind16T)
    sind = _bf16(sind)

    in_maps = []
    for core in range(n_cores):
        subs = [2 * core, 2 * core + 1]
        Xc = np.concatenate([x[u // 4, u % 4 :: 4, :] for u in subs], 0)
        xT = np.ascontiguousarray(Xc.T)  # [feat 1024, tok 1024]
        xT8 = _bf16(np.ascontiguousarray(xT.reshape(8, 128, 1024).transpose(1, 0, 2)))
        in_maps.append(
            {
                "xT": xT8,
                "wq": wq,
                "wk": wk,
                "wo": wo,
                "wv": wv,
                "bq": bq_a,
                "bk": bk_a,
                "bo": bo_a,
                "masks": masks,
                "ind16": ind16,
                "ind16T": ind16T,
                "sind": sind,
            }
        )
    return in_maps


def kernel(x, Wq, bq, Wk, bk, Wv, bv, Wo, bo, _cores=None):
    from concourse.bass_utils import run_bass_kernel_spmd

    x = np.asarray(x, dtype=np.float32)
    B, N, D = x.shape
    n_cores = N_CORES if _cores is None else _cores
    in_maps = _make_in_maps(
        dict(x=x, Wq=Wq, bq=bq, Wk=Wk, bk=bk, Wv=Wv, bv=bv, Wo=Wo, bo=bo), n_cores
    )
    nc = _get_nc()
    res = run_bass_kernel_spmd(nc, in_maps, core_ids=list(range(n_cores)))

    out = np.zeros((B, N, D), np.float32)
    for core in range(n_cores):
        oc = np.asarray(res.results[core]["outT"], dtype=np.float32)
        ocf = oc.transpose(1, 0, 2).reshape(1024, 1024).T  # [tok, feat]
        for i, u in enumerate([2 * core, 2 * core + 1]):
            out[u // 4, u % 4 :: 4, :] = ocf[512 * i : 512 * (i + 1)]
    return out


# revision 4
# speedup vs baseline: 1.0078x; 1.0078x over previous
"""Trainium2 Bass kernel v2 for nn_DilatedMHCABlock.

Same mod-4 subsequence decomposition as the baseline (16 subsequences of 512
tokens with +-8 banded attention, 2 per core, zero collectives), rebuilt
around HW-profile findings:

- bf16 everywhere (f32 PSUM accumulation): halves DVE element cost, halves
  DMA bytes, keeps matmul at 1 cycle/row at any free size.
- Both Q AND K are norm-folded before scores, so exp needs no per-partition
  scale and one [128, 768] activation covers B|C|edge chunks per head.
- One [128, 768] mask multiply per head (masks are host constants).
- AV writes head pairs directly at PSUM partition offsets 0/64 (legal
  offsets are 0/32/64) - no partition-staging DMAs or copies.
- Softmax denominators accumulate into a shared [16, 512] PSUM tile via
  per-head indicator matmuls; normalization is deferred past AV.
- ~40 large contiguous DMAs (vs 245 small ones): weights are host-packed so
  every DMA descriptor is >= 1KB contiguous.
- Scores for iteration k+1 are emitted before AV of iteration k so the
  in-order PE queue never stalls on the exp/mask round trip.
"""
import sys

sys.path.insert(0, "/opt/trn_rl_repo")

import numpy as np

import bass_rust
import concourse.bass as bass
import concourse.mybir as mybir
import concourse.tile as tile

F32 = mybir.dt.float32
BF16 = mybir.dt.bfloat16
EPS = 1e-6
N_CORES = 8
ACT = mybir.ActivationFunctionType


# ---------------------------------------------------------------------------
# walrus wait legalization (same as baseline): at most 1 sync wait per
# instruction (2 on EventSemaphore); split the excess.
_wait_counter = [0]


def _legalize_waits(nc):
    f = nc.m.functions[0]
    for blk in f.blocks:
        insts = blk.instructions
        out = []
        changed = False
        for inst in insts:
            si = inst.sync_info
            waits = list(si.on_wait) if si is not None else []
            cap = 2 if isinstance(inst, mybir.InstEventSemaphore) else 1
            if len(waits) > cap:
                extra, keep = waits[:-cap], waits[-cap:]
                for i in range(0, len(extra), 2):
                    es = mybir.InstEventSemaphore(
                        name=f"wait_split_{_wait_counter[0]}", ins=[], outs=[]
                    )
                    _wait_counter[0] += 1
                    es.engine = inst.engine
                    es.sync_info = bass_rust.SyncInfo(
                        on_wait=extra[i : i + 2], on_update=[]
                    )
                    out.append(es)
                si.on_wait = keep
                changed = True
            out.append(inst)
        if changed:
            blk.instructions = out


# ---------------------------------------------------------------------------
def _build_nc(phases=5):
    nc = bass.Bass()

    xT_d = nc.declare_dram_parameter("xT", [128, 8, 1024], BF16, isOutput=False)
    wq_d = nc.declare_dram_parameter("wq", [8, 128, 8, 128], BF16, isOutput=False)
    wk_d = nc.declare_dram_parameter("wk", [8, 128, 8, 128], BF16, isOutput=False)
    wo_d = nc.declare_dram_parameter("wo", [8, 128, 8, 128], BF16, isOutput=False)
    wv_d = nc.declare_dram_parameter("wv", [2, 128, 8, 512], BF16, isOutput=False)
    bq_d = nc.declare_dram_parameter("bq", [128, 8], F32, isOutput=False)
    bk_d = nc.declare_dram_parameter("bk", [128, 8], F32, isOutput=False)
    bo_d = nc.declare_dram_parameter("bo", [128, 8], F32, isOutput=False)
    mask_d = nc.declare_dram_parameter("masks", [128, 1024], BF16, isOutput=False)
    i16_d = nc.declare_dram_parameter("ind16", [128, 8, 16], BF16, isOutput=False)
    i16t_d = nc.declare_dram_parameter("ind16T", [16, 8, 128], BF16, isOutput=False)
    sind_d = nc.declare_dram_parameter("sind", [128, 16, 16], BF16, isOutput=False)
    out_d = nc.declare_dram_parameter("outT", [128, 8, 1024], BF16, isOutput=True)

    with tile.TileContext(nc) as tc, nc.allow_low_precision(
        reason="bf16 compute within 2e-2 tolerance"
    ):
        _emit(nc, tc, xT_d, wq_d, wk_d, wo_d, wv_d, bq_d, bk_d, bo_d, mask_d,
              i16_d, i16t_d, sind_d, out_d, phases)

    _legalize_waits(nc)
    return nc


def _emit(nc, tc, xT_d, wq_d, wk_d, wo_d, wv_d, bq_d, bk_d, bo_d, mask_d,
          i16_d, i16t_d, sind_d, out_d, phases=5):
    from contextlib import ExitStack

    ctx = ExitStack()
    with ctx:
        p_const = ctx.enter_context(tc.tile_pool(name="const", bufs=1))
        p_x = ctx.enter_context(tc.tile_pool(name="x", bufs=1))
        p_w = ctx.enter_context(tc.tile_pool(name="w", bufs=2))
        p_wo = ctx.enter_context(tc.tile_pool(name="wo", bufs=8))
        p_wv = ctx.enter_context(tc.tile_pool(name="wv", bufs=2))
        p_qk = ctx.enter_context(tc.tile_pool(name="qk", bufs=32))
        p_v = ctx.enter_context(tc.tile_pool(name="v", bufs=8))
        p_at = ctx.enter_context(tc.tile_pool(name="at", bufs=16))
        p_sq = ctx.enter_context(tc.tile_pool(name="sq", bufs=16))
        p_inv = ctx.enter_context(tc.tile_pool(name="inv", bufs=10))
        p_ex = ctx.enter_context(tc.tile_pool(name="ex", bufs=3))
        p_ot = ctx.enter_context(tc.tile_pool(name="ot", bufs=6))
        pp = ctx.enter_context(tc.tile_pool(name="pp", bufs=1, space="PSUM"))
        scb = [pp.tile([128, 1024], F32, tag=f"scb{r}", name=f"scb{r}")
               for r in range(3)]
        sml = pp.tile([128, 1024], F32, tag="sml", name="sml")
        psn = [0]  # rolling counter for [128, 512] matmul psum regions

        def next_ps():
            r = scb[psn[0] % 3][:, 0:512]
            psn[0] += 1
            return r

        # ---- constants ----------------------------------------------------
        bq_sb = p_const.tile([128, 8], F32, tag="bq")
        bk_sb = p_const.tile([128, 8], F32, tag="bk")
        bo_sb = p_const.tile([128, 8], F32, tag="bo")
        nc.gpsimd.dma_start(out=bq_sb, in_=bq_d[:, :])
        nc.gpsimd.dma_start(out=bk_sb, in_=bk_d[:, :])
        nc.gpsimd.dma_start(out=bo_sb, in_=bo_d[:, :])

        m_sb = p_const.tile([128, 1024], BF16, tag="masks")
        nc.gpsimd.dma_start(out=m_sb, in_=mask_d[:, :])
        zq = p_const.tile([128, 64], BF16, tag="zq")
        nc.vector.memset(zq, 0.0)
        ind16 = p_const.tile([128, 8, 16], BF16, tag="ind16")
        nc.gpsimd.dma_start(out=ind16, in_=i16_d[:, :, :])
        ind16T = p_const.tile([16, 8, 128], BF16, tag="ind16T")
        nc.gpsimd.dma_start(out=ind16T, in_=i16t_d[:, :, :])
        sind = p_const.tile([128, 16, 16], BF16, tag="sind")
        nc.gpsimd.dma_start(out=sind, in_=sind_d[:, :, :])

        xT = p_x.tile([128, 8, 1024], BF16, tag="xT")
        for j in range(8):
            nc.scalar.dma_start(out=xT[:, j, :], in_=xT_d[:, j, :])

        # ---- Q/K projections + bias --------------------------------------
        qt = [[None, None] for _ in range(8)]
        kt = [[None, None] for _ in range(8)]
        for w_d, b_sb, dst in ((wk_d, bk_sb, kt), (wq_d, bq_sb, qt)):
            for i in range(8):
                wt = p_w.tile([128, 8, 128], BF16, tag="w")
                nc.sync.dma_start(out=wt, in_=w_d[i])
                ps = [next_ps() for c in range(2)]
                for j in range(8):
                    for c in range(2):
                        nc.tensor.matmul(
                            ps[c], wt[:, j, :],
                            xT[:, j, 512 * c : 512 * c + 512],
                            start=(j == 0), stop=(j == 7),
                        )
                for c in range(2):
                    t = p_qk.tile([128, 512], BF16, tag="qk", name="qk")
                    nc.vector.tensor_scalar_add(t, ps[c], b_sb[:, i : i + 1])
                    dst[i][c] = t

        # ---- norms: squares + reductions ---------------------------------
        # squares: Q on DVE, K on Act (engine balance)
        sqq = [[None, None] for _ in range(8)]
        sqk = [[None, None] for _ in range(8)]
        for i in range(8):
            for c in range(2):
                tq = p_sq.tile([128, 512], BF16, tag="sq", name="tq")
                nc.vector.tensor_mul(tq, qt[i][c], qt[i][c])
                sqq[i][c] = tq
                tk = p_sq.tile([128, 512], BF16, tag="sqk", name="tk")
                nc.scalar.square(tk, kt[i][c])
                sqk[i][c] = tk
        nrm_regions = {
            ("k", 0): sml[64:80, 512:1024],
            ("k", 1): sml[64:80, 0:512],
            ("q", 0): sml[0:16, 512:1024],
            ("q", 1): sml[32:48, 512:1024],
        }
        nrm_ps = {}
        for nm, sq in (("k", sqk), ("q", sqq)):
            for c in range(2):
                p2 = nrm_regions[(nm, c)]
                for i in range(8):
                    nc.tensor.matmul(p2, ind16[:, i, :], sq[i][c],
                                     start=(i == 0), stop=(i == 7))
                nrm_ps[(nm, c)] = p2

        # 1/||v|| = exp(-0.5 * ln(||v||^2)); values are O(10), eps-free
        inv = {}
        for nm in ("k", "q"):
            for c in range(2):
                t = p_inv.tile([16, 512], F32, tag="nrmf", name=f"nf{nm}{c}")
                nc.scalar.activation(out=t, in_=nrm_ps[(nm, c)], func=ACT.Ln)
                tb = p_inv.tile([16, 512], BF16, tag="nrmb", name=f"nb{nm}{c}")
                nc.scalar.activation(out=tb, in_=t, func=ACT.Exp, scale=-0.5)
                inv[(nm, c)] = tb

        # ---- V projection interleaved with norm folding -------------------
        # V-proj matmul groups fill the PE while the DVE chews through the
        # bcast/fold chain, so the fold latency is hidden.
        v = [None] * 8
        wvt = [None, None]
        for co in range(2):
            wvt[co] = p_wv.tile([128, 8, 512], BF16, tag="wv", name=f"wvt{co}")
            nc.sync.dma_start(out=wvt[co], in_=wv_d[co])
        folds = [(c, i, nm) for c in range(2) for i in range(8)
                 for nm in ("k", "q")]
        fold_regions = [sml[:, 0:512], sml[:, 512:1024]]
        fold_n = [0]

        def emit_fold():
            if fold_n[0] >= len(folds):
                return
            c, i, nm = folds[fold_n[0]]
            pb = fold_regions[fold_n[0] % 2]
            fold_n[0] += 1
            dst = kt if nm == "k" else qt
            nc.tensor.matmul(pb, ind16T[:, i, :], inv[(nm, c)],
                             start=True, stop=True)
            nc.vector.tensor_mul(dst[i][c], dst[i][c], pb)

        psv_pend = []
        for g in range(8):
            for co in range(2):
                psv = next_ps()
                for j in range(8):
                    nc.tensor.matmul(
                        psv, xT[:, j, 128 * g : 128 * g + 128],
                        wvt[co][:, j, :], start=(j == 0), stop=(j == 7),
                    )
                emit_fold()
                emit_fold()
                psv_pend.append((g, co, psv))
                if v[g] is None:
                    v[g] = p_v.tile([128, 1024], BF16, tag="v", name=f"v{g}")
                # drain psv copies one group behind to keep psum slots free
                while len(psv_pend) > 2:
                    gg, cc2, pv = psv_pend.pop(0)
                    nc.vector.tensor_copy(
                        out=v[gg][:, 512 * cc2 : 512 * cc2 + 512], in_=pv
                    )
        while psv_pend:
            gg, cc2, pv = psv_pend.pop(0)
            nc.vector.tensor_copy(
                out=v[gg][:, 512 * cc2 : 512 * cc2 + 512], in_=pv
            )
        while fold_n[0] < len(folds):
            emit_fold()

        if phases <= 2:
            ot = p_ot.tile([128, 1024], BF16, tag="ot")
            nc.vector.tensor_copy(out=ot[:, 0:512], in_=qt[0][0])
            nc.vector.tensor_copy(out=ot[:, 512:1024], in_=kt[0][0])
            nc.gpsimd.dma_start(out=out_d[:, 0, :], in_=ot)
            return

        # ---- attention -----------------------------------------------------
        at = [[None, None] for _ in range(8)]
        for ht in range(8):
            for s in range(2):
                at[ht][s] = p_at.tile([128, 512], BF16, tag="at",
                                      name=f"at{ht}_{s}")
        sums_ps = [sml[0:16, 0:512], sml[32:48, 0:512]]

        iters = [(s, h) for s in range(2) for h in range(16)]
        from collections import deque

        # deferred normalization for subsequence s: 1/sum = exp(-ln(sum)),
        # then per-head-pair broadcast + multiply, threaded into the s=1
        # attention stream.
        inv_s = [None, None]

        def emit_inv_s(s):
            tf = p_inv.tile([16, 512], F32, tag="nrmf", name=f"sdf{s}")
            nc.scalar.activation(out=tf, in_=sums_ps[s], func=ACT.Ln)
            tb = p_inv.tile([16, 512], BF16, tag="nrmb", name=f"sdb{s}")
            nc.scalar.activation(out=tb, in_=tf, func=ACT.Exp, scale=-1.0)
            inv_s[s] = tb

        pbn_n = [0]

        def emit_pbn(ht, s):
            # alternate regions so the bcast->mul chain pipelines 2-deep;
            # scb[2] halves are the po slots, free once s=0 copies are done
            # (s=0 pbn) / all copies done (s=1 pbn)
            pbn = (sml[:, 512:1024] if (s == 0 or pbn_n[0] % 2 == 0)
                   else scb[2][:, 0:512])
            pbn_n[0] += 1
            nc.tensor.matmul(pbn, ind16T[:, ht, :], inv_s[s],
                             start=True, stop=True)
            nc.vector.tensor_mul(at[ht][s], at[ht][s], pbn)

        wo_t = [None] * 8

        def emit_oproj(i, c):
            if wo_t[i] is None:
                wo_t[i] = p_wo.tile([128, 8, 128], BF16, tag="wo",
                                    name=f"wo{i}")
                nc.sync.dma_start(out=wo_t[i], in_=wo_d[i])
            ps = next_ps()
            for j in range(8):
                nc.tensor.matmul(ps, wo_t[i][:, j, :], at[j][c],
                                 start=(j == 0), stop=(j == 7))
            ot = p_ot.tile([128, 512], BF16, tag="ot", name="ot")
            nc.scalar.activation(out=ot, in_=ps, func=ACT.Identity,
                                 bias=bo_sb[:, i : i + 1])
            oq = nc.gpsimd if (2 * i + c) % 2 == 0 else nc.sync
            oq.dma_start(
                out=out_d[:, i, 512 * c : 512 * c + 512], in_=ot
            )

        po_slot = [scb[2][:, 0:512], scb[2][:, 512:1024]]

        WINS = (0, 72, 200, 256)

        def emit_av(k2):
            s, h = iters[k2]
            hp = 64 * (h % 2)
            ht = h // 2
            ex = ex_pend[k2]
            po = po_slot[(k2 // 2) % 2]
            # zero-weight matmul initializes this head's 64-row stripe
            nc.tensor.matmul(po[hp : hp + 64, :], zq, ex[:, 0:512],
                             start=True, stop=True)
            for c in range(4):
                w = WINS[c]
                nc.tensor.matmul(
                    po[hp : hp + 64, w : w + 256],
                    v[4 * s + c][:, 64 * h : 64 * h + 64],
                    ex[:, 256 * c : 256 * c + 256],
                    start=False, stop=True,
                )
            if h == 0:
                nc.tensor.matmul(sums_ps[s], zq[:, 0:16], ex[:, 0:512],
                                 start=True, stop=True)
            for c in range(4):
                w = WINS[c]
                nc.tensor.matmul(
                    sums_ps[s][:, w : w + 256], sind[:, h, :],
                    ex[:, 256 * c : 256 * c + 256],
                    start=False, stop=True,
                )
            if hp == 64:
                nc.vector.tensor_copy(out=at[ht][s], in_=po)

        ex_pend = {}
        for k, (s, h) in enumerate(iters):
            hp = 64 * (h % 2)
            ht = h // 2
            scf = scb[k % 2]
            for c in range(4):
                w = WINS[c]
                nc.tensor.matmul(
                    scf[:, 256 * c : 256 * c + 256],
                    kt[ht][s][hp : hp + 64, 128 * c : 128 * c + 128],
                    qt[ht][s][hp : hp + 64, w : w + 256],
                    start=True, stop=True, tile_position=(hp, 0),
                )
            ex = p_ex.tile([128, 1024], BF16, tag="ex", name="ex")
            nc.scalar.activation(out=ex, in_=scf, func=ACT.Exp)
            nc.vector.tensor_mul(ex, ex, m_sb)
            ex_pend[k] = ex
            if k >= 2:
                emit_av(k - 2)
                del ex_pend[k - 2]
            if phases > 3:
                if k == 18:
                    emit_inv_s(0)
                elif 19 <= k <= 22:
                    emit_pbn(2 * (k - 19), 0)
                    emit_pbn(2 * (k - 19) + 1, 0)
        emit_av(30)
        emit_av(31)

        if phases <= 3:
            ot = p_ot.tile([128, 1024], BF16, tag="ot3", name="ot3")
            nc.vector.tensor_copy(out=ot[:, 0:512], in_=at[0][0])
            nc.vector.tensor_copy(out=ot[:, 512:1024], in_=at[0][1])
            nc.gpsimd.dma_start(out=out_d[:, 0, :], in_=ot)
            return

        # ---- epilogue: s=1 normalization, then both O projection halves ---
        emit_inv_s(1)
        for ht in range(8):
            emit_pbn(ht, 1)
        for i in range(8):
            emit_oproj(i, 0)
            emit_oproj(i, 1)


# ---------------------------------------------------------------------------
def _build_masks():
    # [128, 1024]: 4 blocks of 256 query-columns (windows 0/72/200/256).
    k = np.arange(128)[:, None]
    col = np.arange(256)[None, :]
    P0 = (np.abs(k - col) <= 8)
    P1 = (np.abs(k + 56 - col) <= 8)
    P3 = (np.abs(k + 128 - col) <= 8)
    return np.concatenate([P0, P1, P1, P3], axis=1).astype(np.float32)


_NC_CACHE = {}


def _get_nc(phases=5):
    if phases not in _NC_CACHE:
        _NC_CACHE[phases] = _build_nc(phases)
    return _NC_CACHE[phases]


def _bf16(x):
    import ml_dtypes

    return np.asarray(x, dtype=ml_dtypes.bfloat16)


def _make_in_maps(inputs, n_cores):
    x = np.asarray(inputs["x"], dtype=np.float32)
    Wq = np.asarray(inputs["Wq"], dtype=np.float32)
    Wk = np.asarray(inputs["Wk"], dtype=np.float32)
    Wv = np.asarray(inputs["Wv"], dtype=np.float32)
    Wo = np.asarray(inputs["Wo"], dtype=np.float32)
    bq = np.asarray(inputs["bq"], dtype=np.float32)
    bk = np.asarray(inputs["bk"], dtype=np.float32)
    bv = np.asarray(inputs["bv"], dtype=np.float32)
    bo = np.asarray(inputs["bo"], dtype=np.float32)

    def chunk_w(WT):  # [in, out] -> [8 i, 128 p, 8 j, 128 o]
        return np.ascontiguousarray(
            WT.reshape(8, 128, 8, 128).transpose(2, 1, 0, 3)
        )

    wq = _bf16(chunk_w(Wq.T))
    wk = _bf16(chunk_w(Wk.T))
    wo = _bf16(chunk_w(Wo.T))
    wv = _bf16(
        np.ascontiguousarray(Wv.T.reshape(8, 128, 2, 512).transpose(2, 1, 0, 3))
    )
    bo_eff = (bo + Wo @ bv).astype(np.float32)
    bq_a = np.ascontiguousarray(bq.reshape(8, 128).T)
    bk_a = np.ascontiguousarray(bk.reshape(8, 128).T)
    bo_a = np.ascontiguousarray(bo_eff.reshape(8, 128).T)

    masks = _bf16(_build_masks())
    ind16 = np.zeros((128, 8, 16), np.float32)
    for i in range(8):
        ind16[0:64, i, 2 * i] = 1.0
        ind16[64:128, i, 2 * i + 1] = 1.0
    ind16T = np.zeros((16, 8, 128), np.float32)
    for i in range(8):
        ind16T[2 * i, i, 0:64] = 1.0
        ind16T[2 * i + 1, i, 64:128] = 1.0
    sind = np.zeros((128, 16, 16), np.float32)
    for h in range(16):
        sind[:, h, h] = 1.0
    ind16 = _bf16(ind16)
    ind16T = _bf16(ind16T)
    sind = _bf16(sind)

    in_maps = []
    for core in range(n_cores):
        subs = [2 * core, 2 * core + 1]
        Xc = np.concatenate([x[u // 4, u % 4 :: 4, :] for u in subs], 0)
        xT = np.ascontiguousarray(Xc.T)  # [feat 1024, tok 1024]
        xT8 = _bf16(np.ascontiguousarray(xT.reshape(8, 128, 1024).transpose(1, 0, 2)))
        in_maps.append(
            {
                "xT": xT8,
                "wq": wq,
                "wk": wk,
                "wo": wo,
                "wv": wv,
                "bq": bq_a,
                "bk": bk_a,
                "bo": bo_a,
                "masks": masks,
                "ind16": ind16,
                "ind16T": ind16T,
                "sind": sind,
            }
        )
    return in_maps


def kernel(x, Wq, bq, Wk, bk, Wv, bv, Wo, bo, _cores=None):
    from concourse.bass_utils import run_bass_kernel_spmd

    x = np.asarray(x, dtype=np.float32)
    B, N, D = x.shape
    n_cores = N_CORES if _cores is None else _cores
    in_maps = _make_in_maps(
        dict(x=x, Wq=Wq, bq=bq, Wk=Wk, bk=bk, Wv=Wv, bv=bv, Wo=Wo, bo=bo), n_cores
    )
    nc = _get_nc()
    res = run_bass_kernel_spmd(nc, in_maps, core_ids=list(range(n_cores)))

    out = np.zeros((B, N, D), np.float32)
    for core in range(n_cores):
        oc = np.asarray(res.results[core]["outT"], dtype=np.float32)
        ocf = oc.transpose(1, 0, 2).reshape(1024, 1024).T  # [tok, feat]
        for i, u in enumerate([2 * core, 2 * core + 1]):
            out[u // 4, u % 4 :: 4, :] = ocf[512 * i : 512 * (i + 1)]
    return out
